# revision 53
# baseline (speedup 1.0000x reference)
"""Trainium2 Bass kernel for GCN-biased sparse attention (nn_Attention_37589553775245).

Reference computation (per batch b of 8, one NeuronCore each):
    qkv = x @ w_qkv; q,k,v per head (H=8, DH=64)
    attn = softmax(q k^T / sqrt(DH)) + A_hat        (A_hat = D^-1/2 (ceil(adj)+I) D^-1/2)
    out = (attn @ v) @ w_out + b_out

Sharding: pure batch-parallel across the 8 cores (B=8). A_hat is computed on
host (cheap) and replicated; weights replicated. No collectives.

Design (TimelineSim ~116.6us single-shot / ~111us marginal per rep; PE busy
~99us is the bound — measured HW tracks sim at ~1.4x):
  - ALL matmul operands are bf16 (fp32 PSUM accumulation): same PE cost as
    fp32r at free-dim 512 (1 cycle/row) but half the DMA bytes and SBUF
    residency. rel-err ~5e-3 vs the 2e-2 gate. fp8 was evaluated and
    rejected: quantizing q/k makes a per-row systematic score shift softmax
    cannot average away, and v noise is amplified through the A_hat path.
  - scores are computed transposed (sT[j,i] via lhsT=kT, rhs=qT, K=d=64) so
    softmax denominators ride the attn@v matmul via a ones column in the
    augmented V ([v|1]): PSUM out rows 0:64 = head values, row 64 = denom.
    Odd heads SWDGE-shift their bf16 copy to partitions 64:128 to line up
    with their yE slice (DVE lanes cannot cross partitions; PE matmuls can
    only target PSUM partition bases 0/32/64).
  - the reciprocal 1/denom is partition-broadcast via a DRAM bounce for most
    units (2 chained DMAs, fully off the engines, pipelined across units) and
    via a K=1 ones^T@recip PE matmul into the dead rows of the same PSUM bank
    for the final two units, where bounce latency would sit on the critical
    path.
  - 16 attention units (8 heads x 2 i-chunks) run i-chunk-major and
    SOFTWARE-PIPELINED: each unit's first two score blocks are emitted inside
    the previous unit, so attn@v never starts cold on ACT's exp stream (ACT
    is the second-busiest engine at ~78us). Remaining q/k projection tiles,
    A_hat@V units (split around the attention-tail finish), and the chunk-0
    out-projections weave into the stream as PE filler.
  - out-projections for rows 512:768 pre-accumulate their first 3 f-tiles so
    only ~10 matmuls remain after the final merge; dependency-free warmup
    matmuls bridge the initial DMA wait so the PE enters the real stream at
    full p-state.
  - ALL loads ride the ACT HWDGE ring (drained by ~15us), stores + bounce
    DMAs ride the SP ring, and with reps>1 the NEXT rep's tile set is
    allocated and its loads are emitted mid-body of the current rep, so they
    prefetch ~80us early AND reuse DMA-completion semaphores whose previous
    users are early loads rather than end-of-rep stores (avoids a ~15us
    ACT-sequencer stall at the rep boundary). The next rep's q/k head-0/1
    projections are additionally pre-emitted into the current rep's tail as
    dependency-free PE filler under the final exp drain + merge wait, which
    removes the recurring ~2.8us per-rep endgame stall (steady marginal
    ~105-108us vs 116.6us single-shot in TimelineSim). A_hat^T must NOT
    ride the SWDGE ring (a SWDGE DMA occupies the Pool sequencer for its
    whole transfer).
  - PSUM->SBUF copies go on ACT early (projection phase, ACT idle) and DVE
    elsewhere; GPSIMD/Pool cannot access PSUM on real TRN2 (BIR verifier
    rejects it; CoreSim does not catch it).
  - PSUM budget (8 banks): 2 projection/output accumulators + 2x2-bank score
    tiles + 2 attention-output banks.
"""

import os
import sys

import numpy as np

for _p in ("/opt/trn_rl_repo", "/root/.axon_site/_ro/trn_rl_repo"):
    if _p not in sys.path and os.path.isdir(_p):
        sys.path.insert(0, _p)

import ml_dtypes  # noqa: E402

import concourse.bass as bass  # noqa: E402
import concourse.mybir as mybir  # noqa: E402
import concourse.tile as tile  # noqa: E402
from concourse import bacc  # noqa: E402
from concourse.bass_utils import run_bass_kernel_spmd  # noqa: E402

B, N, DIM, H, DH = 8, 1024, 512, 8, 64
F = H * DH          # 512, inner dim
NT = N // 128       # 8 n-tiles (also j-tiles)
DT = DIM // 128     # 4 dim-tiles
FT = F // 128       # 4 f-tiles
NC2 = N // 512      # 2 i-chunks of 512
SCALE = DH ** -0.5

F32 = mybir.dt.float32
BF16 = mybir.dt.bfloat16

_PROGRAM = None
_last_in_maps = None


def _build_program(reps=1, qk_copies_pool=True, o_copy_pool=True,
                   s_bufs=2, o_bufs=2, mm_bufs=2, exps_bufs=6,
                   mmdt="bf16", per2_bufs=2, n_warm=8, small_bufs=3, pe_bc_all=False):
    MDT = {"bf16": BF16, "f32r": mybir.dt.float32r}[mmdt]
    nc = bacc.Bacc("TRN2", target_bir_lowering=False, debug=False, num_devices=8)

    xT_d = nc.dram_tensor("xT", [DIM, N], MDT, kind="ExternalInput")
    wqkv_d = nc.dram_tensor("wqkv", [DIM, 3 * F], MDT, kind="ExternalInput")
    ahatT_d = nc.dram_tensor("ahatT", [N, N], MDT, kind="ExternalInput")
    wout_d = nc.dram_tensor("wout", [F, DIM], MDT, kind="ExternalInput")
    bout_d = nc.dram_tensor("bout", [1, DIM], F32, kind="ExternalInput")
    out_d = nc.dram_tensor("out", [N, DIM], F32, kind="ExternalOutput")

    with tile.TileContext(nc) as tc:
        with (
            nc.allow_low_precision(
                reason="bf16 softmax-normalize/merge; fp32 PSUM accumulation "
                       "everywhere it matters, tolerance is 2e-2"),
            tc.tile_pool(name="per2", bufs=per2_bufs) as per2,
            tc.tile_pool(name="per1", bufs=1) as per1,
            tc.tile_pool(name="exps", bufs=exps_bufs) as exps,
            tc.tile_pool(name="small", bufs=small_bufs) as small,
            tc.tile_pool(name="outs", bufs=3) as outs,
            tc.tile_pool(name="dscr", bufs=4, space="DRAM") as dscr,
            tc.tile_pool(name="ps_mm", bufs=mm_bufs, space="PSUM") as ps_mm,
            tc.tile_pool(name="ps_s", bufs=s_bufs, space="PSUM") as ps_s,
            tc.tile_pool(name="ps_o", bufs=o_bufs, space="PSUM") as ps_o,
        ):
          def alloc_and_load(rep_idx):
            # per-rep input tile set (per2, double-buffered) + its loads.
            # Called MID-body of the previous rep so the next rep's loads
            # dispatch while the DMA-completion semaphores they reuse are
            # still cheap to wait on (their previous users are that rep's
            # own early loads, not its final stores).
            xT = per2.tile([128, DT, N], MDT, name="xT")      # xT[dim, n]
            wqkv = per2.tile([128, DT, 3 * F], MDT, name="wqkv")
            qkT = per2.tile([128, 2 * FT, N], MDT, name="qkT")
            v_sb = per2.tile([128, NT, F], MDT, name="v_sb")  # v[n, f]
            vaug = per2.tile([128, NT, H, DH + 1], MDT, name="vaug")
            wout = per2.tile([128, FT, DIM], MDT, name="wout")
            ahatT = per2.tile([128, NT, N], MDT, name="ahatT")
            bout_bc = per2.tile([128, DIM], F32, name="bout_bc")
            # ones-column init (cheap; its WAR is vs the rep that used this
            # buffer two generations ago, long finished by emission time)
            nc.gpsimd.memset(vaug[:, :, :, DH:DH + 1], 1.0)

            def load_wqkv(fc):
                nc.scalar.dma_start(
                    out=wqkv[:, :, fc * 256:(fc + 1) * 256],
                    in_=wqkv_d[:, fc * 256:(fc + 1) * 256].rearrange(
                        "(t p) f -> p t f", p=128),
                )

            # ALL loads ride the ACT ring (drained by ~15us each rep); the
            # SP ring keeps the mid-rep bounces + stores instead, so neither
            # head-of-line-blocks the other. A_hat^T must NOT ride the
            # SWDGE/Pool queue (a software SWDGE DMA occupies the Pool
            # sequencer for its whole transfer).
            nc.scalar.dma_start(out=xT[:, 0, :], in_=xT_d[0:128, :])
            load_wqkv(0)                 # q cols 0:256 (heads 0..3)
            for dt_i in range(1, DT):
                nc.scalar.dma_start(
                    out=xT[:, dt_i, :],
                    in_=xT_d[dt_i * 128:(dt_i + 1) * 128, :],
                )
            load_wqkv(2)                 # k cols 512:768 (heads 0..3)
            for fc in (4, 5, 1, 3):      # v cols first, rest of q/k
                load_wqkv(fc)
            nc.scalar.dma_start(
                out=wout,
                in_=wout_d[:, :].rearrange("(t p) n -> p t n", p=128),
            )
            nc.scalar.dma_start(
                out=ahatT,
                in_=ahatT_d[:, :].rearrange("(t p) n -> p t n", p=128),
            )
            nc.scalar.dma_start(out=bout_bc,
                                in_=bout_d[0:1, :].to_broadcast((128, DIM)))
            return xT, wqkv, qkT, v_sb, vaug, wout, ahatT, bout_bc

          ones_sb = per1.tile([128, 512], BF16)
          nc.gpsimd.memset(ones_sb[64:65, :], 1.0)
          tiles = alloc_and_load(0)
          qk_pre = False    # did the previous rep emit this rep's qk(0)/qk(4)?
          for _rep in range(reps):
            xT, wqkv, qkT, v_sb, vaug, wout, ahatT, bout_bc = tiles
            yT = per1.tile([128, FT, N], MDT)           # Y^T[f, i] (A_hat V part)
            yE = per1.tile([128, FT, N], MDT)           # Y^T (exp-attention part)

            # PE warmup: dependency-free matmuls bridge the initial DMA wait
            # so the tensor engine enters the real stream already ramped to
            # full p-state instead of idling and restarting at half clock.
            for _w in range(n_warm if _rep == 0 else 0):
                wps = ps_mm.tile([128, 512], F32, tag="mm", name="wps")
                nc.tensor.matmul(wps[0:64, :], ones_sb[64:65, 0:64],
                                 ones_sb[64:65, :])


            # PSUM->SBUF copies: GPSIMD/Pool cannot access PSUM on real
            # TRN2 (BIR verifier rejects it), so spread them over ACT (idle
            # during the projection phase) and DVE.
            def qk_copy(dst, src, eng="act"):
                if eng == "act" and qk_copies_pool:
                    nc.scalar.copy(out=dst, in_=src)
                else:
                    nc.vector.tensor_copy(out=dst, in_=src)

            def dve_copy(dst, src):
                nc.vector.tensor_copy(out=dst, in_=src)

            # ---- phase 1: qT/kT (transposed) and v (natural) -----------
            def emit_qk(ft, eng="act"):
                for c in range(NC2):
                    ps = ps_mm.tile([128, 512], F32, tag="mm")
                    for dt_i in range(DT):
                        nc.tensor.matmul(
                            ps,
                            wqkv[:, dt_i, ft * 128:(ft + 1) * 128],
                            xT[:, dt_i, c * 512:(c + 1) * 512],
                            start=(dt_i == 0),
                            stop=(dt_i == DT - 1),
                        )
                    qk_copy(qkT[:, ft, c * 512:(c + 1) * 512], ps, eng)

            def emit_v(nt_lo=0, nt_hi=NT):
                for nt in range(nt_lo, nt_hi):
                    ps = ps_mm.tile([128, 512], F32, tag="mm")
                    for dt_i in range(DT):
                        nc.tensor.matmul(
                            ps,
                            xT[:, dt_i, nt * 128:(nt + 1) * 128],
                            wqkv[:, dt_i, 2 * F:3 * F],
                            start=(dt_i == 0),
                            stop=(dt_i == DT - 1),
                        )
                    dve_copy(v_sb[:, nt, :], ps)
                    nc.vector.tensor_copy(
                        out=vaug[:, nt, :, 0:DH],
                        in_=ps.rearrange("p (h d) -> p h d", h=H),
                    )

            def ahat_unit(ft, c, mid=None):
                # (A_hat @ V)^T [f-tile ft, i-chunk c] -> yT. `mid` lets the
                # caller interleave work (e.g. the attention tail finish)
                # halfway through the accumulation so merges are not delayed
                # by the full 8-matmul chain.
                ps = ps_mm.tile([128, 512], F32, tag="mm")
                for jt in range(NT):
                    if jt == NT // 2 and mid is not None:
                        mid()
                    nc.tensor.matmul(
                        ps,
                        v_sb[:, jt, ft * 128:(ft + 1) * 128],
                        ahatT[:, jt, c * 512:(c + 1) * 512],
                        start=(jt == 0),
                        stop=(jt == NT - 1),
                    )
                dve_copy(yT[:, ft, c * 512:(c + 1) * 512], ps)

            def attn_tail_pre(h, po):
                # DVE/Pool part of the softmax normalize: copy the exp-attn
                # rows + ridden denominator out of PSUM (bank rows become
                # dead), take the reciprocal, and for odd heads SWDGE-shift
                # the bf16 rows to partitions 64:128 (to line up with their
                # yE slice; DVE lanes cannot cross partitions).
                osb = small.tile([128, 512], BF16, tag="osb")
                nc.vector.tensor_copy(out=osb[0:65, :], in_=po[0:65, :])
                rc = small.tile([128, 512], BF16, tag="recip")
                nc.vector.reciprocal(out=rc[64:65, :], in_=osb[64:65, :])
                if h % 2 == 1:
                    nc.gpsimd.dma_start(out=osb[64:128, :], in_=osb[0:64, :])
                return osb, rc

            def attn_tail_fin(h, c, po, osb, rc, pe_bcast=False):
                # partition-broadcast of the reciprocal. Default: DRAM bounce
                # (2 chained DMAs, ~4us latency but fully off the engines and
                # pipelined across units). pe_bcast: a K=1 PE matmul
                # (ones[1,64]^T @ rc[1,512]) into the now-dead rows of the
                # same PSUM bank (~213ns latency) — used for the final units
                # where the bounce latency would sit on the critical path.
                vlo, vhi = (64, 128) if h % 2 == 1 else (0, 64)
                ysl = yE[vlo:vhi, h // 2, c * 512:(c + 1) * 512]
                if pe_bcast:
                    nc.tensor.matmul(
                        po[vlo:vhi, :],
                        ones_sb[64:65, 0:64],
                        rc[64:65, :],
                    )
                    nc.vector.tensor_mul(ysl, osb[vlo:vhi, :], po[vlo:vhi, :])
                else:
                    scr = dscr.tile([1, 512], BF16, tag="scr")
                    nc.sync.dma_start(out=scr, in_=rc[64:65, :])
                    bcast = small.tile([128, 512], BF16, tag="bcast")
                    nc.sync.dma_start(out=bcast[vlo:vhi, :],
                                      in_=scr.to_broadcast((64, 512)))
                    nc.vector.tensor_mul(ysl, osb[vlo:vhi, :],
                                         bcast[vlo:vhi, :])

            def scores_jb(h, c, jb):
                # one 2-j-tile score block + its exp; returns the et tile
                hb = (h % 2) * 64
                ht = h // 2
                ps_sc = ps_s.tile([128, 2, 512], F32, tag="ps")
                for e in range(2):
                    jt = jb * 2 + e
                    # scoresT[j, i] = sum_d kT[d, j] qT[d, i]
                    nc.tensor.matmul(
                        ps_sc[:, e, :],
                        qkT[hb:hb + 64, FT + ht, jt * 128:(jt + 1) * 128],
                        qkT[hb:hb + 64, ht, c * 512:(c + 1) * 512],
                    )
                et = exps.tile([128, 2, 512], MDT, tag="exp")
                nc.scalar.activation(
                    out=et, in_=ps_sc,
                    func=mybir.ActivationFunctionType.Exp,
                    scale=float(SCALE),
                )
                return et

            def av_jb(h, out_ap, jb, et):
                for e in range(2):
                    jt = jb * 2 + e
                    # [expv^T ; denom] accumulation
                    nc.tensor.matmul(
                        out_ap,
                        vaug[:, jt, h, :],
                        et[:, e, :],
                        start=(jt == 0),
                        stop=(jt == NT - 1),
                    )

            def attn_unit(h, c, weave=None, pe_bcast=False):
                # one head, one 512-wide i-chunk. Score blocks run one jb
                # ahead of the attn@v accumulation so the PE has independent
                # work while ACT computes each exp batch.
                ps_out = ps_o.tile([128, 512], F32, tag="po")
                out_ap = ps_out[0:65, :]
                ets = [scores_jb(h, c, 0), scores_jb(h, c, 1)]
                av_jb(h, out_ap, 0, ets[0])
                ets.append(scores_jb(h, c, 2))
                av_jb(h, out_ap, 1, ets[1])
                ets.append(scores_jb(h, c, 3))
                av_jb(h, out_ap, 2, ets[2])
                av_jb(h, out_ap, 3, ets[3])
                osb, rc = attn_tail_pre(h, ps_out)
                if weave is not None:
                    weave()
                attn_tail_fin(h, c, ps_out, osb, rc, pe_bcast=pe_bcast)

            def merge(ft, c):
                sl = slice(c * 512, (c + 1) * 512)
                nc.vector.tensor_add(yT[:, ft, sl], yT[:, ft, sl], yE[:, ft, sl])

            def outproj(nt):
                ps = ps_mm.tile([128, 512], F32, tag="mm")
                for ft in range(FT):
                    nc.tensor.matmul(
                        ps,
                        yT[:, ft, nt * 128:(nt + 1) * 128],
                        wout[:, ft, :],
                        start=(ft == 0),
                        stop=(ft == FT - 1),
                    )
                ot = outs.tile([128, DIM], F32, tag="ot")
                nc.vector.tensor_add(ot, ps, bout_bc)
                nc.sync.dma_start(out=out_d[nt * 128:(nt + 1) * 128, :], in_=ot)

            # ---- emission schedule -------------------------------------
            if not qk_pre:
                emit_qk(0)    # q heads 0,1
                emit_qk(4)    # k heads 0,1

            # unit (h=0, c=0) emits its score blocks interleaved with emit_v
            # halves so ACT's 64us exp stream starts ~12us earlier; its
            # attn@v runs after emit_v (vaug must precede it in PE order).
            u0_et = [scores_jb(0, 0, 0), scores_jb(0, 0, 1)]
            emit_v(0, NT // 2)
            u0_et += [scores_jb(0, 0, 2), scores_jb(0, 0, 3)]
            emit_v(NT // 2, NT)
            u0_po = ps_o.tile([128, 512], F32, tag="po")
            for jb in range(4):
                av_jb(0, u0_po[0:65, :], jb, u0_et[jb])
            u0_osb, u0_rc = attn_tail_pre(0, u0_po)
            first_ets = [scores_jb(1, 0, 0), scores_jb(1, 0, 1)]
            attn_tail_fin(0, 0, u0_po, u0_osb, u0_rc)  # bounce
            emit_qk(1)        # q heads 2,3 (unit h=2 needs it)
            if _rep + 1 < reps:
                tiles = alloc_and_load(_rep + 1)

            # remaining 15 units, software-pipelined: each unit's first two
            # score blocks (sc0/sc1) are emitted inside the PREVIOUS unit, so
            # a unit's attn@v never starts cold on ACT, and ACT's exp stream
            # stays fed through the endgame. weave_a (qk-tile emission) must
            # precede the next unit's scores; weave_b (A_hat / outproj
            # filler) sits between tail_pre and tail_fin to cover the
            # reciprocal latency.
            rest_qk = [5, 2, 6, 3, 7]
            units = []
            for h in range(1, H):
                units.append(dict(
                    h=h, c=0,
                    weave_a=(lambda ft: (lambda: emit_qk(ft)))(
                        rest_qk[h - 1]) if h - 1 < len(rest_qk) else None,
                    weave_b=(lambda ft: (lambda fin: ahat_unit(
                        ft, 0, mid=lambda: fin.pop()())))(
                        h - 4) if h >= 4 else None,
                    post=(lambda ft: (lambda: merge(ft, 0)))(
                        h - 4) if h >= 4 else None,
                    pe_bcast=pe_bc_all,
                ))
            for h in range(H):
                wb = None
                if h < 3:
                    wb = (lambda nt: (lambda fin: outproj(nt)))(h)
                elif h >= 4:
                    wb = (lambda ft: (lambda fin: ahat_unit(
                        ft, 1, mid=lambda: fin.pop()())))(h - 4)
                units.append(dict(
                    h=h, c=1, weave_a=None, weave_b=wb,
                    post=(lambda ft: (lambda: merge(ft, 1)))(
                        h - 4) if h >= 4 else None,
                    pe_bcast=pe_bc_all or (h >= 6),
                ))

            def emit_units(units, first_ets):
                ets = {0: first_ets}           # unit idx -> [et tiles]
                for i, u in enumerate(units):
                    h, c = u["h"], u["c"]
                    po = ps_o.tile([128, 512], F32, tag="po", name="po")
                    out_ap = po[0:65, :]
                    e = ets.pop(i)
                    av_jb(h, out_ap, 0, e[0])
                    e.append(scores_jb(h, c, 2))
                    av_jb(h, out_ap, 1, e[1])
                    e.append(scores_jb(h, c, 3))
                    if u["weave_a"] is not None:
                        u["weave_a"]()
                    av_jb(h, out_ap, 2, e[2])
                    nxt = units[i + 1] if i + 1 < len(units) else None
                    if nxt is not None:
                        ets[i + 1] = [scores_jb(nxt["h"], nxt["c"], 0)]
                    av_jb(h, out_ap, 3, e[3])
                    if nxt is not None:
                        ets[i + 1].append(scores_jb(nxt["h"], nxt["c"], 1))
                    osb, rc = attn_tail_pre(h, po)
                    fin = [lambda: attn_tail_fin(h, c, po, osb, rc,
                                                 pe_bcast=u["pe_bcast"])]
                    if u["weave_b"] is not None:
                        u["weave_b"](fin)
                    if fin:
                        fin.pop()()
                    if u["post"] is not None:
                        u["post"]()

            emit_units(units, first_ets)

            if _rep + 1 < reps:
                # next rep's first projections: dependency-free PE filler
                # under the last unit's exp drain and merge(3,1) wait.
                nxT, nwqkv, nqkT = tiles[0], tiles[1], tiles[2]
                for ft in (0, 4):
                    for c in range(NC2):
                        ps = ps_mm.tile([128, 512], F32, tag="mm", name="nqk")
                        for dt_i in range(DT):
                            nc.tensor.matmul(
                                ps,
                                nwqkv[:, dt_i, ft * 128:(ft + 1) * 128],
                                nxT[:, dt_i, c * 512:(c + 1) * 512],
                                start=(dt_i == 0),
                                stop=(dt_i == DT - 1),
                            )
                        nc.scalar.copy(
                            out=nqkT[:, ft, c * 512:(c + 1) * 512], in_=ps)
                qk_pre = True

            outproj(3)                       # c0 rows: no merge(.,1) dep
            pp = {}
            for nt in (4, 5):
                pp[nt] = ps_mm.tile([128, 512], F32, tag="mm", name=f"pp{nt}")
                for ft in range(FT - 1):
                    nc.tensor.matmul(
                        pp[nt],
                        yT[:, ft, nt * 128:(nt + 1) * 128],
                        wout[:, ft, :],
                        start=(ft == 0),
                        stop=False,
                    )
            for nt in (4, 5):                # finish after merge(3,1)
                nc.tensor.matmul(
                    pp[nt],
                    yT[:, FT - 1, nt * 128:(nt + 1) * 128],
                    wout[:, FT - 1, :],
                    start=False,
                    stop=True,
                )
                ot = outs.tile([128, DIM], F32, tag="ot")
                nc.vector.tensor_add(ot, pp[nt], bout_bc)
                nc.sync.dma_start(out=out_d[nt * 128:(nt + 1) * 128, :], in_=ot)
            for nt in (6, 7):
                outproj(nt)

    nc.compile()
    return nc


def _get_program():
    global _PROGRAM
    if _PROGRAM is None:
        _PROGRAM = _build_program()
    return _PROGRAM


def kernel(x, adj, w_qkv, w_out, b_out):
    x = np.asarray(x, dtype=np.float32)
    adj = np.asarray(adj, dtype=np.float32)
    w_qkv = np.asarray(w_qkv, dtype=np.float32)
    w_out = np.asarray(w_out, dtype=np.float32)
    b_out = np.asarray(b_out, dtype=np.float32).reshape(1, DIM)

    # host-side: normalized adjacency bias, replicated (cheap: one 1024^2 pass)
    A = np.ceil(adj) + np.eye(N, dtype=np.float32)
    dinv = A.sum(axis=1) ** -0.5
    A_hat = (A * dinv[:, None]) * dinv[None, :]

    bf = ml_dtypes.bfloat16
    ahatT = np.ascontiguousarray(A_hat.T).astype(bf)
    wqkv_b = np.ascontiguousarray(w_qkv).astype(bf)
    wout_b = np.ascontiguousarray(w_out).astype(bf)

    nc = _get_program()
    in_maps = []
    for b in range(B):
        in_maps.append({
            "xT": np.ascontiguousarray(x[b].T).astype(bf),
            "wqkv": wqkv_b,
            "ahatT": ahatT,
            "wout": wout_b,
            "bout": b_out,
        })
    global _last_in_maps
    _last_in_maps = in_maps
    res = run_bass_kernel_spmd(nc, in_maps, list(range(B)))
    out = np.stack([res.results[b]["out"] for b in range(B)], axis=0)
    return out.astype(np.float32)


if __name__ == "__main__":
    rng = np.random.default_rng(0)
    x = rng.standard_normal((B, N, DIM), dtype=np.float32)
    adj = (rng.random((N, N), dtype=np.float32) < 0.05).astype(np.float32) * 0.5
    w_qkv = rng.standard_normal((DIM, 3 * F), dtype=np.float32) * DIM ** -0.5
    w_out = rng.standard_normal((F, DIM), dtype=np.float32) * F ** -0.5
    b_out = np.zeros(DIM, dtype=np.float32)
    out = kernel(x=x, adj=adj, w_qkv=w_qkv, w_out=w_out, b_out=b_out)
    print("out", out.shape, out.dtype, np.abs(out).max())
